# revision 1
# baseline (speedup 1.0000x reference)
"""Trainium2 Bass kernel for CustomAttention (B=4, N=2048, C=1024, H=16).

Sharding: 8-way tensor-parallel over heads (2 heads per core, all batches).
Each core computes qkv projection for its head slice, full attention for its
(batch, head) pairs, and a partial output projection over its 128 channels.
Host sums the 8 partial projections and adds proj_b.

Per-core layouts (host-prepped so no on-device transposes of big tensors):
  xT    [1024, 8192]  x reshaped [B*N, C] and transposed (shared by all cores)
  wqkv  [1024, 384]   qkv weight rows for (q,k,v) x (2 heads x 64) , transposed;
                      q rows pre-scaled by 1/sqrt(D)
  bqkv  [384]         matching bias (q part pre-scaled)
  biasT [2, 2048, 2048]  attn_bias[h][n, m] transposed to [m, n] per head
  pw    [128, 1024]   proj_w columns for this core's 128 channels, transposed
Output:
  outT  [1024, 8192]  partial (x @ Wproj_slice) transposed; host sums, adds
                      proj_b, transposes back.

Attention per (head h, query-chunk nch of 512, batch b):
  scoresT[m, n] tiles [128, 512] = kT_blk.T @ qT_blk  (PSUM)
  p = exp(scoresT + biasT)       (DVE add, ACT exp)
  outPV [65, 512] += v_aug_blk.T @ p   accumulated over 16 m-blocks, where
      v_aug has a ones-column so row 64 is the softmax denominator.
  attn_outT[h*64:(h+1)*64, b, n] = outPV[0:64] * (1/denom) broadcast.
"""

import sys

if "/opt/trn_rl_repo" not in sys.path:
    sys.path.insert(0, "/opt/trn_rl_repo")

import numpy as np

B, N, C, H, D = 4, 2048, 1024, 16, 64
T = B * N  # 8192
HPC = 2  # heads per core
NCORES = 8
MB = N // 128  # 16 key blocks per batch
NCH = N // 512  # 4 query chunks per batch
TC_ = T // 512  # 16 token chunks (qkv phase)
KC = C // 128  # 8 contraction chunks (qkv phase)
JC = C // 128  # 8 output-channel chunks (proj phase)

_CACHE = {}


def build_nc():
    import concourse.bass as bass
    import concourse.bacc as bacc
    import concourse.mybir as mybir
    import concourse.tile as tile
    from concourse.masks import make_identity
    from contextlib import ExitStack

    F32 = mybir.dt.float32
    F32R = mybir.dt.float32r
    EXP = mybir.ActivationFunctionType.Exp
    IDENT = mybir.ActivationFunctionType.Identity

    def r(ap):
        return ap

    nc = bacc.Bacc(None, target_bir_lowering=False)
    xT = nc.dram_tensor("xT", [C, T], F32R, kind="ExternalInput")
    wqkv = nc.dram_tensor("wqkv", [C, 3 * HPC * D], F32R, kind="ExternalInput")
    bqkv = nc.dram_tensor("bqkv", [3 * HPC * D], F32, kind="ExternalInput")
    biasT = nc.dram_tensor("biasT", [HPC, N, N], F32, kind="ExternalInput")
    pw = nc.dram_tensor("pw", [HPC * D, C], F32R, kind="ExternalInput")
    outT = nc.dram_tensor("outT", [C, T], F32, kind="ExternalOutput")

    with tile.TileContext(nc) as tc, ExitStack() as ctx:
        sing = ctx.enter_context(tc.tile_pool(name="sing", bufs=1))
        ps = ctx.enter_context(tc.tile_pool(name="ps", bufs=1, space="PSUM"))

        # ---- constants / residents ----
        b_sb = sing.tile([128, 3], F32)
        nc.sync.dma_start(out=b_sb, in_=bqkv.rearrange("(m p) -> p m", m=3))
        ident = sing.tile([128, 128], F32)
        make_identity(nc, ident)
        pw_sb = sing.tile([128, C], F32R)
        nc.sync.dma_start(out=pw_sb, in_=pw[:, :])

        qT = sing.tile([128, T], F32R)  # rows: q_h0 d0..63 | q_h1 d0..63
        kT = sing.tile([128, T], F32R)
        # v_aug[:, b, mb, :]: cols 0:64 v_h0, col 64 ones, 65:129 v_h1, 129 ones
        v_aug = sing.tile([128, B, MB, 2 * (D + 1)], F32R)
        attn_oT = sing.tile([128, B, N], F32R)
        ones_sb = sing.tile([128, B, MB], F32)
        nc.vector.memset(ones_sb, 1.0)
        nc.vector.tensor_copy(v_aug[:, :, :, D], ones_sb)
        nc.vector.tensor_copy(v_aug[:, :, :, 2 * D + 1], ones_sb)

        # ---- phase 1: qkv projection (output transposed: [384, 8192]) ----
        p1 = tc.alloc_tile_pool(name="p1", bufs=1)
        w_sb = p1.tile([128, KC, 3 * HPC * D], F32R)  # [128, 8, 384]
        nc.sync.dma_start(out=w_sb, in_=wqkv.rearrange("(k p) m -> p k m", p=128))
        for t in range(TC_):
            x_tiles = []
            for kc in range(KC):
                x_t = p1.tile([128, 512], F32R, tag="x", bufs=KC + 2, name=f"x_{t}_{kc}")
                nc.sync.dma_start(
                    out=x_t, in_=xT[kc * 128 : (kc + 1) * 128, t * 512 : (t + 1) * 512]
                )
                x_tiles.append(x_t)
            for m in range(3):
                mm_ps = ps.tile([128, 512], F32, tag="mm", bufs=2, name=f"qkps_{t}_{m}")
                for kc in range(KC):
                    nc.tensor.matmul(
                        mm_ps,
                        r(w_sb[:, kc, m * 128 : (m + 1) * 128]),
                        r(x_tiles[kc]),
                        start=(kc == 0),
                        stop=(kc == KC - 1),
                    )
                if m < 2:
                    dst = qT if m == 0 else kT
                    nc.scalar.activation(
                        dst[:, t * 512 : (t + 1) * 512],
                        mm_ps,
                        IDENT,
                        bias=b_sb[:, m : m + 1],
                    )
                else:
                    vstg = p1.tile([128, 512], F32, tag="vstg", bufs=2, name=f"vs_{t}")
                    nc.scalar.activation(vstg, mm_ps, IDENT, bias=b_sb[:, 2:3])
                    b_idx, off = divmod(t, TC_ // B)
                    for j in range(4):
                        mb = off * 4 + j
                        tr_ps = ps.tile(
                            [128, 128], F32, tag="tr", bufs=2, name=f"tr_{t}_{j}"
                        )
                        nc.tensor.transpose(
                            tr_ps, vstg[:, j * 128 : (j + 1) * 128], ident
                        )
                        nc.vector.tensor_copy(
                            v_aug[:, b_idx, mb, 0:D], tr_ps[:, 0:D]
                        )
                        nc.vector.tensor_copy(
                            v_aug[:, b_idx, mb, D + 1 : 2 * D + 1], tr_ps[:, D : 2 * D]
                        )

        # ---- phase 2: attention ----
        p1.release()
        work = ctx.enter_context(tc.tile_pool(name="work", bufs=1))
        for h in range(HPC):
            hd = h * D
            for nch in range(NCH):
                n0 = nch * 512
                out_pss = [
                    ps.tile(
                        [D + 1, 512], F32, tag=f"opv{b}", bufs=1, name=f"opv_{h}_{nch}_{b}"
                    )
                    for b in range(B)
                ]
                for mb in range(MB):
                    m0 = mb * 128
                    bias_t = work.tile(
                        [128, 512], F32, tag="bias", bufs=3, name=f"bias_{h}_{nch}_{mb}"
                    )
                    nc.sync.dma_start(
                        out=bias_t, in_=biasT[h, m0 : m0 + 128, n0 : n0 + 512]
                    )
                    for b in range(B):
                        bo = b * N
                        s_ps = ps.tile(
                            [128, 512], F32, tag="mm", bufs=2, name=f"s_{h}_{nch}_{mb}_{b}"
                        )
                        nc.tensor.matmul(
                            s_ps,
                            r(kT[hd : hd + D, bo + m0 : bo + m0 + 128]),
                            r(qT[hd : hd + D, bo + n0 : bo + n0 + 512]),
                            start=True,
                            stop=True,
                        )
                        p_sb = work.tile(
                            [128, 512], F32, tag="p", bufs=3, name=f"p_{h}_{nch}_{mb}_{b}"
                        )
                        nc.vector.tensor_add(p_sb, s_ps, bias_t)
                        e_sb = work.tile(
                            [128, 512], F32R, tag="e", bufs=3, name=f"e_{h}_{nch}_{mb}_{b}"
                        )
                        nc.scalar.activation(e_sb, p_sb, EXP)
                        nc.tensor.matmul(
                            out_pss[b],
                            r(v_aug[:, b, mb, h * (D + 1) : (h + 1) * (D + 1)]),
                            r(e_sb),
                            start=(mb == 0),
                            stop=(mb == MB - 1),
                        )
                for b in range(B):
                    # denominator lives at PSUM partition 64 (row D)
                    den = work.tile([D + 1, 512], F32, tag="den", bufs=1, name=f"dn_{h}_{nch}_{b}")
                    nc.vector.tensor_copy(den[D : D + 1, :], out_pss[b][D : D + 1, :])
                    den0 = work.tile([1, 512], F32, tag="den0", bufs=2, name=f"d0_{h}_{nch}_{b}")
                    nc.sync.dma_start(out=den0, in_=den[D : D + 1, :])
                    dbc = work.tile([D, 512], F32, tag="dbc", bufs=2, name=f"db_{h}_{nch}_{b}")
                    nc.gpsimd.partition_broadcast(dbc, den0)
                    rbc = work.tile([D, 512], F32, tag="rbc", bufs=2, name=f"rb_{h}_{nch}_{b}")
                    nc.vector.reciprocal(rbc, dbc)
                    if h == 0:
                        nc.vector.tensor_mul(
                            attn_oT[0:D, b, n0 : n0 + 512], out_pss[b][0:D, :], rbc
                        )
                    else:
                        tmp = work.tile(
                            [D, 512], F32R, tag="tmp", bufs=2, name=f"tm_{h}_{nch}_{b}"
                        )
                        nc.vector.tensor_mul(tmp, out_pss[b][0:D, :], rbc)
                        nc.sync.dma_start(
                            out=attn_oT[D : 2 * D, b, n0 : n0 + 512], in_=tmp
                        )

        # ---- phase 3: partial output projection ----
        for b in range(B):
            bo = b * N
            for jc in range(JC):
                for ncq in range(NCH):
                    n0 = ncq * 512
                    pr_ps = ps.tile(
                        [128, 512], F32, tag="mm", bufs=2, name=f"pr_{b}_{jc}_{ncq}"
                    )
                    nc.tensor.matmul(
                        pr_ps,
                        r(pw_sb[:, jc * 128 : (jc + 1) * 128]),
                        r(attn_oT[:, b, n0 : n0 + 512]),
                        start=True,
                        stop=True,
                    )
                    o_sb = work.tile(
                        [128, 512], F32, tag="o", bufs=2, name=f"o_{b}_{jc}_{ncq}"
                    )
                    nc.vector.tensor_copy(o_sb, pr_ps)
                    nc.sync.dma_start(
                        out=outT[jc * 128 : (jc + 1) * 128, bo + n0 : bo + n0 + 512],
                        in_=o_sb,
                    )

    nc.compile()
    return nc


def _get_nc():
    if "nc" not in _CACHE:
        _CACHE["nc"] = build_nc()
    return _CACHE["nc"]


def make_in_maps(x, attn_bias, qkv_w, qkv_b, proj_w):
    x = np.ascontiguousarray(np.asarray(x, dtype=np.float32))
    attn_bias = np.asarray(attn_bias, dtype=np.float32)
    qkv_w = np.asarray(qkv_w, dtype=np.float32)
    qkv_b = np.asarray(qkv_b, dtype=np.float32)
    proj_w = np.asarray(proj_w, dtype=np.float32)

    xT = np.ascontiguousarray(x.reshape(T, C).T)
    biasT_full = np.ascontiguousarray(attn_bias[0].transpose(0, 2, 1))
    scale = 1.0 / np.sqrt(D)

    in_maps = []
    for cid in range(NCORES):
        h0 = HPC * cid
        rows = np.r_[h0 * D : (h0 + 1) * D, (h0 + 1) * D : (h0 + 2) * D]
        wq = qkv_w[rows, :] * scale
        wk = qkv_w[C + rows, :]
        wv = qkv_w[2 * C + rows, :]
        wqkv_c = np.ascontiguousarray(np.concatenate([wq, wk, wv], 0).T)
        bq = qkv_b[rows] * scale
        bk = qkv_b[C + rows]
        bv = qkv_b[2 * C + rows]
        bqkv_c = np.ascontiguousarray(np.concatenate([bq, bk, bv], 0))
        biasT_c = np.ascontiguousarray(biasT_full[h0 : h0 + HPC])
        pw_c = np.ascontiguousarray(proj_w[:, cid * 128 : (cid + 1) * 128].T)
        in_maps.append(
            {"xT": xT, "wqkv": wqkv_c, "bqkv": bqkv_c, "biasT": biasT_c, "pw": pw_c}
        )
    return in_maps


def combine_outputs(partials, proj_b):
    proj_b = np.asarray(proj_b, dtype=np.float32)
    acc = partials[0].copy()
    for p in partials[1:]:
        acc += p
    out = acc.T + proj_b[None, :]
    return np.ascontiguousarray(out.reshape(B, N, C).astype(np.float32))


def kernel(x, attn_bias, qkv_w, qkv_b, proj_w, proj_b):
    from concourse.bass_utils import run_bass_kernel_spmd

    in_maps = make_in_maps(x, attn_bias, qkv_w, qkv_b, proj_w)
    res = run_bass_kernel_spmd(_get_nc(), in_maps, core_ids=list(range(NCORES)))
    partials = [res.results[i]["outT"] for i in range(NCORES)]
    return combine_outputs(partials, proj_b)

